# revision 29
# baseline (speedup 1.0000x reference)
"""GCN (2-layer + linear classifier) on 8 Trainium2 NeuronCores.

Math: with A = adjacency+self-loops and dis = deg^-1/2 (deg over incoming
edges incl. self-loops), PyG gcn_norm gives norm_e = dis[src]*dis[dst], which
is separable. So each conv layer is
    out = dis ⊙ (A_binary @ ((dis ⊙ h) @ W)) + b
and since the aggregation is linear it commutes with the W matmul:
layer 1 aggregates RAW x-tilde rows (dis ⊙ x, a host input) and applies W1 to
each window's 128-column aggregate afterwards — the replicated h1-table build
phase of earlier versions disappears entirely.

Distribution (8 cores): nodes split into 8 contiguous chunks; edges
partitioned by destination owner (segment-sum is local); the single
cross-core exchange is one fp16 AllGather of the layer-2 message table
(dis ⊙ relu-activated h2 ⊙ dis @ W2 rows), issued in chunks overlapped with
the layer-1 gather tail, then staged Shared->local DRAM.

Per core, aggregation runs per 128-destination-node window: source rows are
fetched from a row-major DRAM table with dma_gather (≤7 128-row tiles per
call, 4 SWDGE queues), reduced onto a PSUM accumulator with TensorEngine
matmuls against one-hot fp8 selection matrices built by DVE is_equal, then
the epilogue (W1 matmul for layer 1, dis-scale, bias, relu, W2/classifier
matmul) runs on DVE/ACT/PE. Tables are split at row 32768 into lo/hi views
because dma_gather indices are int16. Both layers share one table geometry
(padded row-major node ids), so the idx/wrow metadata is loaded once.
"""
import os
import numpy as np

import concourse.bacc as bacc
import concourse.bass as bass
import concourse.mybir as mybir
import concourse.tile as tile
from concourse import library_config
from concourse.bass_utils import run_bass_kernel_spmd

N_CORES = 8
D = 128           # feature dim (= hidden dim = partition count)
LO_DEFAULT = 32768

fp16 = mybir.dt.float16
fp8 = mybir.dt.float8e4
f32 = mybir.dt.float32
i16 = mybir.dt.int16


# ---------------------------------------------------------------- host prep

def _wrap16(v):
    """dma_gather index layout: idx i -> partition i%16, col i//16,
    replicated across all eight 16-partition groups."""
    a = v.reshape(-1, 16).T.astype(np.int16)
    return np.tile(a, (8, 1))


def prep(x, edge_index, n_cores=N_CORES, lo_rows=LO_DEFAULT):
    N = x.shape[0]
    locN = N // n_cores
    assert locN * n_cores == N
    WPC = -(-locN // 128)              # real (dst) windows per core
    NLOC = -(-locN // 512) * 512       # padded nodes per core (512-aligned)
    NPAD = n_cores * NLOC
    assert lo_rows % 128 == 0 and lo_rows < 32768 + 1

    src_all = np.asarray(edge_index[0]).astype(np.int64)
    dst_all = np.asarray(edge_index[1]).astype(np.int64)

    # degree includes the implicit self-loop; the loops themselves are NOT in
    # the gather lists — each window's self-loop block is read directly from
    # the own-chunk rows and applied via an identity matmul
    deg = (np.bincount(dst_all, minlength=N) + 1).astype(np.float32)

    d_core = dst_all // locN
    d_off = dst_all - d_core * locN
    w_global = d_core * WPC + d_off // 128
    wrow = (d_off % 128).astype(np.float16)
    spid = (src_all // locN) * NLOC + (src_all % locN)   # padded row-major id
    hi_flag = (spid >= lo_rows).astype(np.int64)

    key = w_global * 2 + hi_flag
    order = np.argsort(key, kind="stable")
    counts = np.bincount(key, minlength=n_cores * WPC * 2).reshape(n_cores, WPC, 2)
    offs = np.concatenate([[0], np.cumsum(counts.reshape(-1))]).astype(np.int64)

    # shared (max-across-cores) tile structure
    T_lo = [int(-(-counts[:, w, 0].max() // 128)) for w in range(WPC)]
    T_hi = [int(-(-counts[:, w, 1].max() // 128)) for w in range(WPC)]

    spid_sorted = spid[order]
    wrow_sorted = wrow[order]

    per_core = []
    for c in range(n_cores):
        ilo_parts, ihi_parts, wr_parts = [], [], []
        for w in range(WPC):
            base = (c * WPC + w) * 2
            for h, T in ((0, T_lo[w]), (1, T_hi[w])):
                n = T * 128
                if n == 0:
                    continue
                a, b = offs[base + h], offs[base + h + 1]
                sp = spid_sorted[a:b]
                wr = wrow_sorted[a:b]
                pad = n - (b - a)
                idx = np.concatenate([sp - (lo_rows if h else 0),
                                      np.zeros(pad, np.int64)]).astype(np.int16)
                wrc = np.concatenate([wr, np.full(pad, -1.0, np.float16)])
                (ihi_parts if h else ilo_parts).append(idx)
                wr_parts.append(wrc.reshape(T, 128).T)
        idx_lo = _wrap16(np.concatenate(ilo_parts)) if ilo_parts else np.zeros((128, 8), np.int16)
        idx_hi = _wrap16(np.concatenate(ihi_parts)) if ihi_parts else np.zeros((128, 8), np.int16)
        wrow_c = np.concatenate(wr_parts, axis=1).astype(np.float16)

        # per-core dis row over its padded local nodes (pads get dis 1),
        # replicated across the 128 partitions host-side
        dr = np.ones((NLOC,), np.float32)
        dr[:locN] = 1.0 / np.sqrt(deg[c * locN:(c + 1) * locN])
        disrep_c = np.tile(dr[None, :], (128, 1)).astype(np.float32)
        per_core.append(dict(idx_lo=idx_lo, idx_hi=idx_hi, wrow=wrow_c,
                             disrep=disrep_c))

    # x-tilde padded row-major, fp16: (dis * x) at row spid; pads zero
    dis = 1.0 / np.sqrt(deg)
    xs = (np.asarray(x, np.float32) * dis[:, None]).astype(np.float16)
    xrows = np.zeros((NPAD, D), np.float16)
    for c in range(n_cores):
        xrows[c * NLOC: c * NLOC + locN] = xs[c * locN:(c + 1) * locN]

    for c in range(n_cores):
        per_core[c]["xown"] = np.ascontiguousarray(
            xrows[c * NLOC:(c + 1) * NLOC])

    struct = dict(N=N, locN=locN, WPC=WPC, NLOC=NLOC, NPAD=NPAD,
                  lo_rows=lo_rows, T_lo=tuple(T_lo), T_hi=tuple(T_hi),
                  n_cores=n_cores)
    return struct, per_core, xrows


# ------------------------------------------------------------- bass program

def build(struct):
    WPC, NLOC, NPAD = struct["WPC"], struct["NLOC"], struct["NPAD"]
    LO = struct["lo_rows"]
    T_lo, T_hi = struct["T_lo"], struct["T_hi"]
    n_cores = struct["n_cores"]
    CL = max(8, 8 * sum(T_lo))
    CH = max(8, 8 * sum(T_hi))
    TT = sum(T_lo) + sum(T_hi)
    maxT = max(T_lo[w] + T_hi[w] for w in range(WPC))

    nc = bacc.Bacc("TRN2", target_bir_lowering=False, debug=False,
                   num_devices=n_cores, num_swdge_queues=4,
                   dynamic_dma_scratch_size=49152)
    xrows_d = nc.dram_tensor("xrows", [NPAD, D], fp16, kind="ExternalInput")
    xown_d = nc.dram_tensor("xown", [NLOC, D], fp16, kind="ExternalInput")
    W1_d = nc.dram_tensor("W1", [D, D], fp16, kind="ExternalInput")
    W2_d = nc.dram_tensor("W2", [D, D], fp16, kind="ExternalInput")
    Wc_d = nc.dram_tensor("Wc", [D, 2], fp16, kind="ExternalInput")
    b1_d = nc.dram_tensor("b1c", [D, 1], f32, kind="ExternalInput")
    b2_d = nc.dram_tensor("b2c", [D, 1], f32, kind="ExternalInput")
    bc_d = nc.dram_tensor("bcrep", [D, 2], f32, kind="ExternalInput")
    iota_d = nc.dram_tensor("iota", [D, D], fp16, kind="ExternalInput")
    ident_d = nc.dram_tensor("ident", [D, D], fp16, kind="ExternalInput")
    disrep_d = nc.dram_tensor("disrep", [D, NLOC], f32, kind="ExternalInput")
    ilo_d = nc.dram_tensor("idx_lo", [128, CL], i16, kind="ExternalInput")
    ihi_d = nc.dram_tensor("idx_hi", [128, CH], i16, kind="ExternalInput")
    wrow_d = nc.dram_tensor("wrow", [128, TT], fp16, kind="ExternalInput")
    out_d = nc.dram_tensor("out", [NLOC, 2], f32, kind="ExternalOutput")

    ag_in = nc.dram_tensor("ag_in", [NLOC, D], fp16)
    htab2l = nc.dram_tensor("htab2l", [NPAD, D], fp16)

    with tile.TileContext(nc) as tc:
        nc.gpsimd.load_library(library_config.mlp)
        with (
            tc.tile_pool(name="const", bufs=1) as cp,
            tc.tile_pool(name="work", bufs=3) as wp,
            tc.tile_pool(name="msgp", bufs=2) as mp,
            tc.tile_pool(name="Sp", bufs=4) as sp_,
            tc.tile_pool(name="psum", bufs=2, space="PSUM") as pp,
        ):
            # ---- constants
            W1s = cp.tile([D, D], fp16)
            W2s = cp.tile([D, D], fp16)
            Wcs = cp.tile([D, 2], fp16)
            nc.sync.dma_start(out=W1s[:], in_=W1_d[:])
            nc.sync.dma_start(out=W2s[:], in_=W2_d[:])
            nc.sync.dma_start(out=Wcs[:], in_=Wc_d[:])
            ident = cp.tile([D, D], fp16)
            nc.sync.dma_start(out=ident[:], in_=ident_d[:])
            b1c = cp.tile([D, 1], f32)
            b2c = cp.tile([D, 1], f32)
            bcr = cp.tile([D, 2], f32)
            iota = cp.tile([D, D], fp16)
            nc.sync.dma_start(out=b1c[:], in_=b1_d[:])
            nc.sync.dma_start(out=b2c[:], in_=b2_d[:])
            nc.sync.dma_start(out=bcr[:], in_=bc_d[:])
            nc.sync.dma_start(out=iota[:], in_=iota_d[:])
            ilo = cp.tile([128, CL], i16)
            ihi = cp.tile([128, CH], i16)
            wro = cp.tile([128, TT], fp16)
            nc.sync.dma_start(out=ilo[:], in_=ilo_d[:])
            nc.sync.dma_start(out=ihi[:], in_=ihi_d[:])
            nc.sync.dma_start(out=wro[:], in_=wrow_d[:])

            # replicated dis rows, precomputed host-side
            disrep = cp.tile([128, NLOC], f32)
            nc.scalar.dma_start(out=disrep[:], in_=disrep_d[:])

            # ---- one aggregation layer over all windows
            CLO = [8 * sum(T_lo[:w]) for w in range(WPC)]
            CHI = [8 * sum(T_hi[:w]) for w in range(WPC)]
            CT = [sum(T_lo[:w]) + sum(T_hi[:w]) for w in range(WPC)]
            WORDER = list(range(WPC))

            def layer(tab_lo, tab_hi, own_rows, emit_window, post_window=None):
                qn = [0]
                for w in WORDER:
                    tl, th = T_lo[w], T_hi[w]
                    Tw = tl + th
                    clo, chi, ct = CLO[w], CHI[w], CT[w]
                    msg = mp.tile([128, maxT, 128], fp16, tag="msg", bufs=5)
                    # single_packet coalesces a gather's descriptor stream into
                    # one SDMA packet (much better 256B-descriptor throughput);
                    # packets cap at 64 descriptors = 7 tiles per dma_gather
                    GMAX = 7
                    for t0 in range(0, tl, GMAX):
                        tc_ = min(GMAX, tl - t0)
                        nc.gpsimd.dma_gather(
                            msg[:, t0:t0 + tc_, :], tab_lo,
                            ilo[:, clo + t0 * 8:clo + (t0 + tc_) * 8],
                            tc_ * 128, tc_ * 128, D, queue_num=qn[0] % 4)
                        qn[0] += 1
                    for t0 in range(0, th, GMAX):
                        tc_ = min(GMAX, th - t0)
                        nc.gpsimd.dma_gather(
                            msg[:, tl + t0:tl + t0 + tc_, :], tab_hi,
                            ihi[:, chi + t0 * 8:chi + (t0 + tc_) * 8],
                            tc_ * 128, tc_ * 128, D, queue_num=qn[0] % 4)
                        qn[0] += 1
                    mself = mp.tile([128, 128], fp16, tag="mself", bufs=6)
                    # scalar (not sync): the sync FIFO carries AG collectives +
                    # staging, whose input waits must not stall window loads
                    nc.scalar.dma_start(out=mself[:],
                                        in_=own_rows[w * 128:(w + 1) * 128, :])
                    pa = pp.tile([128, 128], f32, space="PSUM", tag="agg", bufs=3)
                    nc.tensor.matmul(out=pa[:], lhsT=mself[:], rhs=ident[:],
                                     start=True, stop=(Tw == 0))
                    SG = 8
                    for g0 in range(0, Tw, SG):
                        gk = min(SG, Tw - g0)
                        S = sp_.tile([128, SG * 128], fp8, tag="S", bufs=6)
                        iap = iota[:]
                        iota_b = bass.AP(iap.tensor, iap.offset,
                                         [iap.ap[0], [0, gk], iap.ap[1]])
                        nc.vector.tensor_tensor(
                            out=S[:, :gk * 128].rearrange("p (t d) -> p t d", t=gk),
                            in0=wro[:, ct + g0:ct + g0 + gk].to_broadcast([128, gk, 128]),
                            in1=iota_b, op=mybir.AluOpType.is_equal)
                        for t in range(g0, g0 + gk):
                            ts_ = t - g0
                            nc.tensor.matmul(
                                out=pa[:], lhsT=msg[:, t, :],
                                rhs=S[:, ts_ * 128:(ts_ + 1) * 128],
                                start=False, stop=(t == Tw - 1))
                    emit_window(w, pa)
                    if post_window is not None:
                        post_window(w)

            # layer 1 window epilogue: agg0 is the raw x-tilde aggregate;
            # h1 = agg0^T@W1 (as [f2, dst] via W1 stationary);
            # h2 = relu(dis*h1 + b1); y = dis*h2; htilde2 = y^T @ W2 -> ag_in
            def epi1(w, pa):
                dw = disrep[:, w * 128:(w + 1) * 128]
                y0 = wp.tile([128, 128], fp16, tag="y0")
                nc.scalar.activation(y0[:], pa[:],
                                     mybir.ActivationFunctionType.Identity)
                p1w = pp.tile([128, 128], f32, space="PSUM", tag="mm", bufs=3)
                nc.tensor.matmul(out=p1w[:], lhsT=W1s[:], rhs=y0[:],
                                 start=True, stop=True)
                z = wp.tile([128, 128], f32, tag="z")
                nc.vector.tensor_mul(out=z[:], in0=p1w[:], in1=dw)
                h2 = wp.tile([128, 128], f32, tag="h2")
                nc.scalar.activation(h2[:], z[:], mybir.ActivationFunctionType.Relu,
                                     bias=b1c[:, 0:1], scale=1.0)
                y = wp.tile([128, 128], fp16, tag="y")
                nc.vector.tensor_mul(out=y[:], in0=h2[:], in1=dw)
                p2 = pp.tile([128, 128], f32, space="PSUM", tag="mm", bufs=3)
                nc.tensor.matmul(out=p2[:], lhsT=y[:], rhs=W2s[:], start=True, stop=True)
                hb = wp.tile([128, 128], fp16, tag="hb")
                nc.scalar.activation(hb[:], p2[:],
                                     mybir.ActivationFunctionType.Identity)
                nc.scalar.dma_start(out=ag_in[w * 128:(w + 1) * 128, :], in_=hb[:])

            # chunked AllGather over 128-row (window) granular chunks of the
            # REAL windows only (htab2l pad rows are never gathered), issued
            # several windows after the last contributing epilogue so the
            # CollectiveCompute (which must ride the gpsimd FIFO) never stalls
            # the gather stream at the FIFO head; the collective and the
            # staging copy into local DRAM (Shared-space gathers are slow)
            # overlap the layer-1 gather tail. The tail chunk is small so the
            # post-last-window serial tail is short.
            spl = sorted({0, min(12, WPC), min(24, WPC), min(34, WPC),
                          min(43, WPC), WPC})

            def emit_ag(k, final=False):
                a, b = spl[k] * 128, spl[k + 1] * 128
                if a == b:
                    return
                agk = nc.dram_tensor(f"ag_out{k}", [n_cores * (b - a), D], fp16,
                                     addr_space="Shared")
                nc.gpsimd.collective_compute(
                    "AllGather", mybir.AluOpType.bypass,
                    replica_groups=[list(range(n_cores))],
                    ins=[ag_in[a:b, :].opt()], outs=[agk.ap().opt()])
                for c in range(n_cores):
                    # mid-layer staging stays off scalar (epilogues live
                    # there); the final chunk runs post-layer, so both HWDGE
                    # engines can share it
                    eng = nc.scalar if (final and c % 2) else nc.sync
                    eng.dma_start(
                        out=htab2l[c * NLOC + a:c * NLOC + b, :],
                        in_=agk[c * (b - a):(c + 1) * (b - a), :])

            ag_last = {}
            for k in range(len(spl) - 1):
                last_w = min(WPC, spl[k + 1]) - 1
                ag_last.setdefault(min(last_w + 5, WPC - 1), []).append(k)

            def post1(w):
                for k in ag_last.get(w, []):
                    emit_ag(k, final=(w == WORDER[-1]))

            with nc.named_scope("agg1"):
                layer(xrows_d[0:LO, :], xrows_d[LO:, :], xown_d, epi1,
                      post_window=post1)

            # layer 2 window epilogue: out3 = dis*agg + b2 ; out = out3^T@Wc + bc
            outacc = cp.tile([128, WPC, 2], f32)

            def epi2(w, pa):
                dw = disrep[:, w * 128:(w + 1) * 128]
                z = wp.tile([128, 128], f32, tag="z2")
                nc.vector.tensor_mul(out=z[:], in0=pa[:], in1=dw)
                o3 = wp.tile([128, 128], fp16, tag="o3")
                nc.scalar.activation(o3[:], z[:], mybir.ActivationFunctionType.Identity,
                                     bias=b2c[:, 0:1], scale=1.0)
                p3 = pp.tile([128, 2], f32, space="PSUM", tag="cls")
                nc.tensor.matmul(out=p3[:], lhsT=o3[:], rhs=Wcs[:], start=True, stop=True)
                nc.vector.tensor_add(out=outacc[:, w, :], in0=p3[:], in1=bcr[:])

            with nc.named_scope("agg2"):
                layer(htab2l[0:LO, :], htab2l[LO:, :], ag_in, epi2)
                nc.sync.dma_start(
                    out=out_d[:WPC * 128, :].rearrange("(w p) c -> p w c", p=128),
                    in_=outacc[:])

    nc.compile()
    return nc


# ------------------------------------------------------------------ driver

_CACHE = {}


def _get_program(struct):
    key = tuple(sorted((k, v) for k, v in struct.items()))
    if key not in _CACHE:
        _CACHE[key] = build(struct)
    return _CACHE[key]


def kernel(x, edge_index, W1, b1, W2, b2, Wc, bc):
    x = np.asarray(x)
    N = x.shape[0]
    struct, per_core, xrows = prep(x, edge_index)
    nc = _get_program(struct)
    locN, NLOC = struct["locN"], struct["NLOC"]

    common = dict(
        xrows=xrows,
        W1=np.asarray(W1, np.float16),
        W2=np.asarray(W2, np.float16),
        Wc=np.asarray(Wc, np.float16),
        b1c=np.asarray(b1, np.float32).reshape(D, 1),
        b2c=np.asarray(b2, np.float32).reshape(D, 1),
        bcrep=np.tile(np.asarray(bc, np.float32).reshape(1, 2), (D, 1)),
        iota=np.tile(np.arange(D, dtype=np.float16), (D, 1)),
        ident=np.eye(D, dtype=np.float16),
    )
    in_maps = []
    for c in range(N_CORES):
        m = dict(common)
        m["disrep"] = per_core[c]["disrep"]
        m["xown"] = per_core[c]["xown"]
        m["idx_lo"] = per_core[c]["idx_lo"]
        m["idx_hi"] = per_core[c]["idx_hi"]
        m["wrow"] = per_core[c]["wrow"]
        in_maps.append(m)

    trace = bool(int(os.environ.get("KERNEL_TRACE", "0")))
    res = run_bass_kernel_spmd(nc, in_maps, core_ids=list(range(N_CORES)),
                               trace=trace,
                               tmpdir=os.environ.get("KERNEL_TRACE_DIR"))
    if trace and res.exec_time_ns is not None:
        print(f"HW exec time: {res.exec_time_ns} ns", flush=True)
        if res.per_core_scope_times:
            for k, v in res.per_core_scope_times.items():
                print(f"  scope {k}: {v}", flush=True)

    out = np.empty((N, 2), np.float32)
    for c in range(N_CORES):
        out[c * locN:(c + 1) * locN] = res.results[c]["out"][:locN]
    return out


# revision 32
# speedup vs baseline: 1.0242x; 1.0242x over previous
"""GCN (2-layer + linear classifier) on 8 Trainium2 NeuronCores.

Math: with A = adjacency+self-loops and dis = deg^-1/2 (deg over incoming
edges incl. self-loops), PyG gcn_norm gives norm_e = dis[src]*dis[dst], which
is separable. So each conv layer is
    out = dis ⊙ (A_binary @ ((dis ⊙ h) @ W)) + b
and since the aggregation is linear it commutes with the W matmul:
layer 1 aggregates RAW x-tilde rows (dis ⊙ x, a host input) and applies W1 to
each window's 128-column aggregate afterwards — the replicated h1-table build
phase of earlier versions disappears entirely.

Distribution (8 cores): nodes split into 8 contiguous chunks; edges
partitioned by destination owner (segment-sum is local); the single
cross-core exchange is one fp16 AllGather of the layer-2 message table
(dis ⊙ relu-activated h2 ⊙ dis @ W2 rows), issued in chunks overlapped with
the layer-1 gather tail, then staged Shared->local DRAM.

Per core, aggregation runs per 128-destination-node window: source rows are
fetched from a row-major DRAM table with dma_gather (≤7 128-row tiles per
call, 4 SWDGE queues), reduced onto a PSUM accumulator with TensorEngine
matmuls against one-hot fp8 selection matrices built by DVE is_equal, then
the epilogue (W1 matmul for layer 1, dis-scale, bias, relu, W2/classifier
matmul) runs on DVE/ACT/PE. Tables are split at row 32768 into lo/hi views
because dma_gather indices are int16. Both layers share one table geometry
(padded row-major node ids), so the idx/wrow metadata is loaded once.
"""
import os
import numpy as np

import concourse.bacc as bacc
import concourse.bass as bass
import concourse.mybir as mybir
import concourse.tile as tile
from concourse import library_config
from concourse.bass_utils import run_bass_kernel_spmd

N_CORES = 8
D = 128           # feature dim (= hidden dim = partition count)
LO_DEFAULT = 32768

fp16 = mybir.dt.float16
fp8 = mybir.dt.float8e4
f32 = mybir.dt.float32
i16 = mybir.dt.int16


# ---------------------------------------------------------------- host prep

def _wrap16(v):
    """dma_gather index layout: idx i -> partition i%16, col i//16,
    replicated across all eight 16-partition groups."""
    a = v.reshape(-1, 16).T.astype(np.int16)
    return np.tile(a, (8, 1))


def prep(x, edge_index, n_cores=N_CORES, lo_rows=LO_DEFAULT):
    N = x.shape[0]
    locN = N // n_cores
    assert locN * n_cores == N
    WPC = -(-locN // 128)              # real (dst) windows per core
    NLOC = -(-locN // 512) * 512       # padded nodes per core (512-aligned)
    NPAD = n_cores * NLOC
    assert lo_rows % 128 == 0 and lo_rows < 32768 + 1

    src_all = np.asarray(edge_index[0]).astype(np.int64)
    dst_all = np.asarray(edge_index[1]).astype(np.int64)

    # degree includes the implicit self-loop; the loops themselves are NOT in
    # the gather lists — each window's self-loop block is read directly from
    # the own-chunk rows and applied via an identity matmul
    deg = (np.bincount(dst_all, minlength=N) + 1).astype(np.float32)

    d_core = dst_all // locN
    d_off = dst_all - d_core * locN
    w_global = d_core * WPC + d_off // 128
    wrow = (d_off % 128).astype(np.float16)
    spid = (src_all // locN) * NLOC + (src_all % locN)   # padded row-major id
    hi_flag = (spid >= lo_rows).astype(np.int64)

    key = w_global * 2 + hi_flag
    order = np.argsort(key, kind="stable")
    counts = np.bincount(key, minlength=n_cores * WPC * 2).reshape(n_cores, WPC, 2)
    offs = np.concatenate([[0], np.cumsum(counts.reshape(-1))]).astype(np.int64)

    # shared (max-across-cores) tile structure
    T_lo = [int(-(-counts[:, w, 0].max() // 128)) for w in range(WPC)]
    T_hi = [int(-(-counts[:, w, 1].max() // 128)) for w in range(WPC)]

    spid_sorted = spid[order]
    wrow_sorted = wrow[order]

    per_core = []
    for c in range(n_cores):
        ilo_parts, ihi_parts, wr_parts = [], [], []
        for w in range(WPC):
            base = (c * WPC + w) * 2
            for h, T in ((0, T_lo[w]), (1, T_hi[w])):
                n = T * 128
                if n == 0:
                    continue
                a, b = offs[base + h], offs[base + h + 1]
                sp = spid_sorted[a:b]
                wr = wrow_sorted[a:b]
                pad = n - (b - a)
                idx = np.concatenate([sp - (lo_rows if h else 0),
                                      np.zeros(pad, np.int64)]).astype(np.int16)
                wrc = np.concatenate([wr, np.full(pad, -1.0, np.float16)])
                (ihi_parts if h else ilo_parts).append(idx)
                wr_parts.append(wrc.reshape(T, 128).T)
        idx_lo = _wrap16(np.concatenate(ilo_parts)) if ilo_parts else np.zeros((128, 8), np.int16)
        idx_hi = _wrap16(np.concatenate(ihi_parts)) if ihi_parts else np.zeros((128, 8), np.int16)
        wrow_c = np.concatenate(wr_parts, axis=1).astype(np.float16)

        # per-core dis row over its padded local nodes (pads get dis 1),
        # replicated across the 128 partitions host-side
        dr = np.ones((NLOC,), np.float32)
        dr[:locN] = 1.0 / np.sqrt(deg[c * locN:(c + 1) * locN])
        disrep_c = np.tile(dr[None, :], (128, 1)).astype(np.float32)
        per_core.append(dict(idx_lo=idx_lo, idx_hi=idx_hi, wrow=wrow_c,
                             disrep=disrep_c))

    # x-tilde padded row-major, fp16: (dis * x) at row spid; pads zero
    dis = 1.0 / np.sqrt(deg)
    xs = (np.asarray(x, np.float32) * dis[:, None]).astype(np.float16)
    xrows = np.zeros((NPAD, D), np.float16)
    for c in range(n_cores):
        xrows[c * NLOC: c * NLOC + locN] = xs[c * locN:(c + 1) * locN]

    for c in range(n_cores):
        per_core[c]["xown"] = np.ascontiguousarray(
            xrows[c * NLOC:(c + 1) * NLOC])

    struct = dict(N=N, locN=locN, WPC=WPC, NLOC=NLOC, NPAD=NPAD,
                  lo_rows=lo_rows, T_lo=tuple(T_lo), T_hi=tuple(T_hi),
                  n_cores=n_cores)
    return struct, per_core, xrows


# ------------------------------------------------------------- bass program

def build(struct):
    WPC, NLOC, NPAD = struct["WPC"], struct["NLOC"], struct["NPAD"]
    LO = struct["lo_rows"]
    T_lo, T_hi = struct["T_lo"], struct["T_hi"]
    n_cores = struct["n_cores"]
    CL = max(8, 8 * sum(T_lo))
    CH = max(8, 8 * sum(T_hi))
    TT = sum(T_lo) + sum(T_hi)
    maxT = max(T_lo[w] + T_hi[w] for w in range(WPC))

    nc = bacc.Bacc("TRN2", target_bir_lowering=False, debug=False,
                   num_devices=n_cores, num_swdge_queues=4,
                   dynamic_dma_scratch_size=49152)
    xrows_d = nc.dram_tensor("xrows", [NPAD, D], fp16, kind="ExternalInput")
    xown_d = nc.dram_tensor("xown", [NLOC, D], fp16, kind="ExternalInput")
    W1_d = nc.dram_tensor("W1", [D, D], fp16, kind="ExternalInput")
    W2_d = nc.dram_tensor("W2", [D, D], fp16, kind="ExternalInput")
    Wc_d = nc.dram_tensor("Wc", [D, 2], fp16, kind="ExternalInput")
    b1_d = nc.dram_tensor("b1c", [D, 1], f32, kind="ExternalInput")
    b2_d = nc.dram_tensor("b2c", [D, 1], f32, kind="ExternalInput")
    bc_d = nc.dram_tensor("bcrep", [D, 2], f32, kind="ExternalInput")
    iota_d = nc.dram_tensor("iota", [D, D], fp16, kind="ExternalInput")
    ident_d = nc.dram_tensor("ident", [D, D], fp16, kind="ExternalInput")
    disrep_d = nc.dram_tensor("disrep", [D, NLOC], f32, kind="ExternalInput")
    ilo_d = nc.dram_tensor("idx_lo", [128, CL], i16, kind="ExternalInput")
    ihi_d = nc.dram_tensor("idx_hi", [128, CH], i16, kind="ExternalInput")
    wrow_d = nc.dram_tensor("wrow", [128, TT], fp16, kind="ExternalInput")
    out_d = nc.dram_tensor("out", [NLOC, 2], f32, kind="ExternalOutput")

    ag_in = nc.dram_tensor("ag_in", [NLOC, D], fp16)
    htab2l = nc.dram_tensor("htab2l", [NPAD, D], fp16)

    with tile.TileContext(nc) as tc:
        nc.gpsimd.load_library(library_config.mlp)
        with (
            tc.tile_pool(name="const", bufs=1) as cp,
            tc.tile_pool(name="work", bufs=3) as wp,
            tc.tile_pool(name="msgp", bufs=2) as mp,
            tc.tile_pool(name="Sp", bufs=4) as sp_,
            tc.tile_pool(name="psum", bufs=2, space="PSUM") as pp,
        ):
            # ---- constants; gather metadata first — the first gather call
            # gates on ilo, everything else is needed only ~40us in
            ilo = cp.tile([128, CL], i16)
            ihi = cp.tile([128, CH], i16)
            wro = cp.tile([128, TT], fp16)
            nc.sync.dma_start(out=ilo[:], in_=ilo_d[:])
            nc.sync.dma_start(out=ihi[:], in_=ihi_d[:])
            nc.sync.dma_start(out=wro[:], in_=wrow_d[:])
            W1s = cp.tile([D, D], fp16)
            W2s = cp.tile([D, D], fp16)
            Wcs = cp.tile([D, 2], fp16)
            ident = cp.tile([D, D], fp16)
            nc.sync.dma_start(out=ident[:], in_=ident_d[:])
            nc.sync.dma_start(out=W1s[:], in_=W1_d[:])
            nc.sync.dma_start(out=W2s[:], in_=W2_d[:])
            nc.sync.dma_start(out=Wcs[:], in_=Wc_d[:])
            b1c = cp.tile([D, 1], f32)
            b2c = cp.tile([D, 1], f32)
            bcr = cp.tile([D, 2], f32)
            iota = cp.tile([D, D], fp16)
            nc.sync.dma_start(out=b1c[:], in_=b1_d[:])
            nc.sync.dma_start(out=b2c[:], in_=b2_d[:])
            nc.sync.dma_start(out=bcr[:], in_=bc_d[:])
            nc.sync.dma_start(out=iota[:], in_=iota_d[:])

            # replicated dis rows, precomputed host-side
            disrep = cp.tile([128, NLOC], f32)
            nc.scalar.dma_start(out=disrep[:], in_=disrep_d[:])

            # ---- one aggregation layer over all windows
            CLO = [8 * sum(T_lo[:w]) for w in range(WPC)]
            CHI = [8 * sum(T_hi[:w]) for w in range(WPC)]
            CT = [sum(T_lo[:w]) + sum(T_hi[:w]) for w in range(WPC)]
            WORDER = list(range(WPC))

            def layer(tab_lo, tab_hi, own_rows, emit_window, post_window=None):
                qn = [0]
                for w in WORDER:
                    tl, th = T_lo[w], T_hi[w]
                    Tw = tl + th
                    clo, chi, ct = CLO[w], CHI[w], CT[w]
                    msg = mp.tile([128, maxT, 128], fp16, tag="msg", bufs=5)
                    # single_packet coalesces a gather's descriptor stream into
                    # one SDMA packet (much better 256B-descriptor throughput);
                    # packets cap at 64 descriptors = 7 tiles per dma_gather
                    GMAX = 7
                    for t0 in range(0, tl, GMAX):
                        tc_ = min(GMAX, tl - t0)
                        nc.gpsimd.dma_gather(
                            msg[:, t0:t0 + tc_, :], tab_lo,
                            ilo[:, clo + t0 * 8:clo + (t0 + tc_) * 8],
                            tc_ * 128, tc_ * 128, D, queue_num=qn[0] % 4)
                        qn[0] += 1
                    for t0 in range(0, th, GMAX):
                        tc_ = min(GMAX, th - t0)
                        nc.gpsimd.dma_gather(
                            msg[:, tl + t0:tl + t0 + tc_, :], tab_hi,
                            ihi[:, chi + t0 * 8:chi + (t0 + tc_) * 8],
                            tc_ * 128, tc_ * 128, D, queue_num=qn[0] % 4)
                        qn[0] += 1
                    mself = mp.tile([128, 128], fp16, tag="mself", bufs=6)
                    # scalar (not sync): the sync FIFO carries AG collectives +
                    # staging, whose input waits must not stall window loads
                    nc.scalar.dma_start(out=mself[:],
                                        in_=own_rows[w * 128:(w + 1) * 128, :])
                    pa = pp.tile([128, 128], f32, space="PSUM", tag="agg", bufs=3)
                    nc.tensor.matmul(out=pa[:], lhsT=mself[:], rhs=ident[:],
                                     start=True, stop=(Tw == 0))
                    SG = 8
                    for g0 in range(0, Tw, SG):
                        gk = min(SG, Tw - g0)
                        S = sp_.tile([128, SG * 128], fp8, tag="S", bufs=6)
                        iap = iota[:]
                        iota_b = bass.AP(iap.tensor, iap.offset,
                                         [iap.ap[0], [0, gk], iap.ap[1]])
                        nc.vector.tensor_tensor(
                            out=S[:, :gk * 128].rearrange("p (t d) -> p t d", t=gk),
                            in0=wro[:, ct + g0:ct + g0 + gk].to_broadcast([128, gk, 128]),
                            in1=iota_b, op=mybir.AluOpType.is_equal)
                        for t in range(g0, g0 + gk):
                            ts_ = t - g0
                            nc.tensor.matmul(
                                out=pa[:], lhsT=msg[:, t, :],
                                rhs=S[:, ts_ * 128:(ts_ + 1) * 128],
                                start=False, stop=(t == Tw - 1))
                    emit_window(w, pa)
                    if post_window is not None:
                        post_window(w)

            # layer 1 window epilogue: agg0 is the raw x-tilde aggregate;
            # h1 = agg0^T@W1 (as [f2, dst] via W1 stationary);
            # h2 = relu(dis*h1 + b1); y = dis*h2; htilde2 = y^T @ W2 -> ag_in
            def epi1(w, pa):
                dw = disrep[:, w * 128:(w + 1) * 128]
                y0 = wp.tile([128, 128], fp16, tag="y0")
                nc.scalar.activation(y0[:], pa[:],
                                     mybir.ActivationFunctionType.Identity)
                p1w = pp.tile([128, 128], f32, space="PSUM", tag="mm", bufs=3)
                nc.tensor.matmul(out=p1w[:], lhsT=W1s[:], rhs=y0[:],
                                 start=True, stop=True)
                z = wp.tile([128, 128], f32, tag="z")
                nc.vector.tensor_mul(out=z[:], in0=p1w[:], in1=dw)
                h2 = wp.tile([128, 128], f32, tag="h2")
                nc.scalar.activation(h2[:], z[:], mybir.ActivationFunctionType.Relu,
                                     bias=b1c[:, 0:1], scale=1.0)
                y = wp.tile([128, 128], fp16, tag="y")
                nc.vector.tensor_mul(out=y[:], in0=h2[:], in1=dw)
                p2 = pp.tile([128, 128], f32, space="PSUM", tag="mm", bufs=3)
                nc.tensor.matmul(out=p2[:], lhsT=y[:], rhs=W2s[:], start=True, stop=True)
                hb = wp.tile([128, 128], fp16, tag="hb")
                nc.scalar.activation(hb[:], p2[:],
                                     mybir.ActivationFunctionType.Identity)
                nc.scalar.dma_start(out=ag_in[w * 128:(w + 1) * 128, :], in_=hb[:])

            # chunked AllGather over 128-row (window) granular chunks of the
            # REAL windows only (htab2l pad rows are never gathered), issued
            # several windows after the last contributing epilogue so the
            # CollectiveCompute (which must ride the gpsimd FIFO) never stalls
            # the gather stream at the FIFO head; the collective and the
            # staging copy into local DRAM (Shared-space gathers are slow)
            # overlap the layer-1 gather tail. The tail chunk is small so the
            # post-last-window serial tail is short.
            spl = sorted({0, min(12, WPC), min(24, WPC), min(36, WPC),
                          min(44, WPC), WPC})

            def emit_ag(k, final=False):
                a, b = spl[k] * 128, spl[k + 1] * 128
                if a == b:
                    return
                agk = nc.dram_tensor(f"ag_out{k}", [n_cores * (b - a), D], fp16,
                                     addr_space="Shared")
                nc.gpsimd.collective_compute(
                    "AllGather", mybir.AluOpType.bypass,
                    replica_groups=[list(range(n_cores))],
                    ins=[ag_in[a:b, :].opt()], outs=[agk.ap().opt()])
                for c in range(n_cores):
                    # mid-layer staging stays off scalar (epilogues live
                    # there); the final chunk runs post-layer, so both HWDGE
                    # engines can share it
                    eng = nc.scalar if (final and c % 2) else nc.sync
                    eng.dma_start(
                        out=htab2l[c * NLOC + a:c * NLOC + b, :],
                        in_=agk[c * (b - a):(c + 1) * (b - a), :])

            ag_last = {}
            for k in range(len(spl) - 1):
                last_w = min(WPC, spl[k + 1]) - 1
                ag_last.setdefault(min(last_w + 5, WPC - 1), []).append(k)

            def post1(w):
                for k in ag_last.get(w, []):
                    emit_ag(k, final=(w == WORDER[-1]))

            with nc.named_scope("agg1"):
                layer(xrows_d[0:LO, :], xrows_d[LO:, :], xown_d, epi1,
                      post_window=post1)

            # layer 2 window epilogue: out3 = dis*agg + b2 ; out = out3^T@Wc + bc
            # stored per-window so the final store isn't one serial tail DMA
            def epi2(w, pa):
                dw = disrep[:, w * 128:(w + 1) * 128]
                z = wp.tile([128, 128], f32, tag="z2")
                nc.vector.tensor_mul(out=z[:], in0=pa[:], in1=dw)
                o3 = wp.tile([128, 128], fp16, tag="o3")
                nc.scalar.activation(o3[:], z[:], mybir.ActivationFunctionType.Identity,
                                     bias=b2c[:, 0:1], scale=1.0)
                p3 = pp.tile([128, 2], f32, space="PSUM", tag="cls")
                nc.tensor.matmul(out=p3[:], lhsT=o3[:], rhs=Wcs[:], start=True, stop=True)
                ow = wp.tile([128, 2], f32, tag="ow", bufs=4)
                nc.vector.tensor_add(out=ow[:], in0=p3[:], in1=bcr[:])
                nc.scalar.dma_start(out=out_d[w * 128:(w + 1) * 128, :], in_=ow[:])

            with nc.named_scope("agg2"):
                layer(htab2l[0:LO, :], htab2l[LO:, :], ag_in, epi2)

    nc.compile()
    return nc


# ------------------------------------------------------------------ driver

_CACHE = {}


def _get_program(struct):
    key = tuple(sorted((k, v) for k, v in struct.items()))
    if key not in _CACHE:
        _CACHE[key] = build(struct)
    return _CACHE[key]


def kernel(x, edge_index, W1, b1, W2, b2, Wc, bc):
    x = np.asarray(x)
    N = x.shape[0]
    struct, per_core, xrows = prep(x, edge_index)
    nc = _get_program(struct)
    locN, NLOC = struct["locN"], struct["NLOC"]

    common = dict(
        xrows=xrows,
        W1=np.asarray(W1, np.float16),
        W2=np.asarray(W2, np.float16),
        Wc=np.asarray(Wc, np.float16),
        b1c=np.asarray(b1, np.float32).reshape(D, 1),
        b2c=np.asarray(b2, np.float32).reshape(D, 1),
        bcrep=np.tile(np.asarray(bc, np.float32).reshape(1, 2), (D, 1)),
        iota=np.tile(np.arange(D, dtype=np.float16), (D, 1)),
        ident=np.eye(D, dtype=np.float16),
    )
    in_maps = []
    for c in range(N_CORES):
        m = dict(common)
        m["disrep"] = per_core[c]["disrep"]
        m["xown"] = per_core[c]["xown"]
        m["idx_lo"] = per_core[c]["idx_lo"]
        m["idx_hi"] = per_core[c]["idx_hi"]
        m["wrow"] = per_core[c]["wrow"]
        in_maps.append(m)

    trace = bool(int(os.environ.get("KERNEL_TRACE", "0")))
    res = run_bass_kernel_spmd(nc, in_maps, core_ids=list(range(N_CORES)),
                               trace=trace,
                               tmpdir=os.environ.get("KERNEL_TRACE_DIR"))
    if trace and res.exec_time_ns is not None:
        print(f"HW exec time: {res.exec_time_ns} ns", flush=True)
        if res.per_core_scope_times:
            for k, v in res.per_core_scope_times.items():
                print(f"  scope {k}: {v}", flush=True)

    out = np.empty((N, 2), np.float32)
    for c in range(N_CORES):
        out[c * locN:(c + 1) * locN] = res.results[c]["out"][:locN]
    return out
